# revision 14
# baseline (speedup 1.0000x reference)
"""Center loss kernel for Trainium2, 8 NeuronCores, data-parallel over batch.

loss = sum((x - centers[labels])**2) / 2 / BATCH

Primary path (v7): algebraic split
    loss*2*B = sum(x^2) - 2*sum_m S_m.c_m + sum_m count_m*||c_m||^2
where rows are sorted by label on the host (any permutation is valid) and
sharded 8 ways so each core's 2048 rows touch <= 128 distinct classes (a
rotation offset of the sorted order is searched to make that hold), m ranges
over the core's local classes and S = per-class row sums of x.

Per core the device then only needs:
  - stream x as fp8-e3m4 [128, 16*2048] (4.2 MB) on the sync ring, with the
    0.26 MB fp8 one-hot lhsT + first chunk on the parallel scalar ring,
  - 16 one-hot matmuls accumulating S[class, feat] into PSUM (TensorE, the
    critical path: ~216 ns per warm N=512 matmul),
  - sum(x^2) in pair-fused ops split between ScalarE (Square+accum) and
    VectorE (scalar_tensor_tensor (x*1)*x with accum),
  - one tail scalar_tensor_tensor (S * C * -2) and tiny final ops,
  - 0.26 MB center slice + counts late on the scalar HWDGE ring.
HBM traffic per core ~4.8 MB vs 17 MB for the v5 gather scheme and 32 MB for
the naive per-row gather. fp8-e3m4 quantization of x/centers costs ~1.8e-4
rel err (gate is 2e-2); X_FP8=False gives a bf16 stream at ~8e-6 instead.

Fallback paths when the class-packing precondition fails: v5 (sorted tiles,
<=16 distinct centers per 128-row tile) then v2 (any label distribution).

Each core returns 128x3 partial sums; the host adds them in f64 and applies
the 1/(2*BATCH) scale.
"""

import numpy as np
import ml_dtypes

import concourse.bacc as bacc
import concourse.bass as bass
import concourse.mybir as mybir
import concourse.tile as tile
from concourse.bass_utils import run_bass_kernel_spmd

N_CORES = 8
BATCH = 16384
FEAT = 2048
NUM_CLASSES = 1000
B_SHARD = BATCH // N_CORES  # 2048
P = 128
N_TILES = B_SHARD // P  # 16
U = 16  # max distinct centers per 128-row tile (sorted labels, v5)
NJ = FEAT // 512  # matmul free-dim chunks
NCHUNKS = 8  # x-stream DMA chunks (v7)


X_FP8 = True  # stream x/oh/cb as fp8-e3m4 (rel err ~1.5e-4) vs bf16 (~2e-6)


def _build_v7():
    xdt = mybir.dt.float8e3 if X_FP8 else mybir.dt.bfloat16
    nc = bacc.Bacc("TRN2", num_devices=N_CORES)
    xq = nc.dram_tensor("xq", [P, N_TILES * FEAT], xdt, kind="ExternalInput").ap()
    oh = nc.dram_tensor("oh", [P, N_TILES * P], xdt, kind="ExternalInput").ap()
    cb = nc.dram_tensor("cb", [P, FEAT], xdt, kind="ExternalInput").ap()
    cnt = nc.dram_tensor("cnt", [P, 1], mybir.dt.float32, kind="ExternalInput").ap()
    out = nc.dram_tensor("out", [P, 3], mybir.dt.float32, kind="ExternalOutput").ap()

    # x-stream chunking (in tiles): small first chunks so compute starts
    # earlier; chunk 0 rides the scalar ring in parallel with chunk 1 on
    # the sync ring. All metadata uses >=512B-per-partition descriptors
    # except cnt, which is only needed at the very end.
    chunk_tiles = [1, 1, 2, 2, 2, 2, 2, 2, 2]
    assert sum(chunk_tiles) == N_TILES
    # squares: pair-fused instructions amortize per-op overhead; singles for
    # the two ramp tiles. (engine, start_tile, ntiles)
    squares = [
        ("v", 0, 1),
        ("a", 1, 1),
        ("a", 2, 2),
        ("v", 4, 2),
        ("a", 6, 2),
        ("v", 8, 2),
        ("a", 10, 2),
        ("v", 12, 2),
        ("a", 14, 2),
    ]

    with tile.TileContext(nc) as tc:
        with (
            tc.tile_pool(name="persist", bufs=1) as ppool,
            tc.tile_pool(name="psum", bufs=1, space="PSUM") as psum_pool,
        ):
            oh_s = ppool.tile([P, N_TILES * P], xdt)
            nc.scalar.dma_start(out=oh_s[:], in_=oh)

            xbuf = ppool.tile([P, N_TILES * FEAT], xdt)
            t0 = 0
            for k, ntk in enumerate(chunk_tiles):
                dma = nc.scalar.dma_start if k == 0 else nc.sync.dma_start
                dma(
                    out=xbuf[:, t0 * FEAT : (t0 + ntk) * FEAT],
                    in_=xq[:, t0 * FEAT : (t0 + ntk) * FEAT],
                )
                t0 += ntk
            # cb/cnt after the stream in ring order; needed only mid/late
            cb_s = ppool.tile([P, FEAT], xdt)
            nc.scalar.dma_start(out=cb_s[:], in_=cb)
            cnt_s = ppool.tile([P, 1], mybir.dt.float32)
            nc.scalar.dma_start(out=cnt_s[:], in_=cnt)

            S = psum_pool.tile([P, FEAT], mybir.dt.float32)
            qacc = ppool.tile([P, N_TILES], mybir.dt.float32)
            nc.vector.memset(qacc[:], 0.0)
            sa = ppool.tile([P, 2 * FEAT], mybir.dt.bfloat16)  # ACT scratch
            sv = ppool.tile([P, 2 * FEAT], mybir.dt.bfloat16)  # DVE scratch

            sq = {t: (eng, nt) for eng, t, nt in squares}
            for t in range(N_TILES):
                ot = oh_s[:, t * P : (t + 1) * P]
                for j in range(NJ):
                    js = slice(j * 512, (j + 1) * 512)
                    nc.tensor.matmul(
                        out=S[:, js],
                        lhsT=ot,
                        rhs=xbuf[:, t * FEAT + j * 512 : t * FEAT + (j + 1) * 512],
                        start=(t == 0),
                        stop=(t == N_TILES - 1),
                    )
                if t not in sq:
                    continue
                eng, nt = sq[t]
                xt = xbuf[:, t * FEAT : (t + nt) * FEAT]
                if eng == "a":
                    nc.scalar.activation(
                        out=sa[:, : nt * FEAT],
                        in_=xt,
                        func=mybir.ActivationFunctionType.Square,
                        accum_out=qacc[:, t : t + 1],
                    )
                else:
                    # fused square+row-sum: out=(x*1)*x, accum_out=sum(out)
                    # (native TENSOR_TENSOR_REDUCE crashes the exec unit on
                    # this runtime; InstTensorScalarPtr works)
                    nc.vector.scalar_tensor_tensor(
                        out=sv[:, : nt * FEAT],
                        in0=xt,
                        scalar=1.0,
                        in1=xt,
                        op0=mybir.AluOpType.mult,
                        op1=mybir.AluOpType.mult,
                        accum_out=qacc[:, t : t + 1],
                    )
            # per-class center norms (after ACT's squares)
            cnw = ppool.tile([P, 1], mybir.dt.float32)
            nc.scalar.activation(
                out=sa[:, :FEAT],
                in_=cb_s[:],
                func=mybir.ActivationFunctionType.Square,
                accum_out=cnw[:],
            )
            partials = ppool.tile([P, 3], mybir.dt.float32)
            # off-critical-path finals first (DVE FIFO order)
            nc.vector.tensor_reduce(
                out=partials[:, 0:1],
                in_=qacc[:],
                axis=mybir.AxisListType.X,
                op=mybir.AluOpType.add,
            )
            nc.vector.tensor_tensor(
                out=partials[:, 2:3],
                in0=cnw[:],
                in1=cnt_s[:],
                op=mybir.AluOpType.mult,
            )
            # tail: cross term -2 * sum_n S[m,n]*c[m,n]
            psc = ppool.tile([P, FEAT], mybir.dt.bfloat16)
            nc.vector.scalar_tensor_tensor(
                out=psc[:],
                in0=S[:],
                scalar=-2.0,
                in1=cb_s[:],
                op0=mybir.AluOpType.mult,
                op1=mybir.AluOpType.mult,
                accum_out=partials[:, 1:2],
            )
            nc.scalar.dma_start(out=out, in_=partials[:])
    nc.finalize()
    return nc


def _build_v5():
    nc = bacc.Bacc("TRN2", num_devices=N_CORES)
    x = nc.dram_tensor("x", [B_SHARD, FEAT], mybir.dt.float32, kind="ExternalInput").ap()
    chi = nc.dram_tensor(
        "chi", [NUM_CLASSES, FEAT], mybir.dt.bfloat16, kind="ExternalInput"
    ).ap()
    uniq = nc.dram_tensor("uniq", [U, N_TILES], mybir.dt.int32, kind="ExternalInput").ap()
    # mall[u, t*128 + i] = one-hot slot of row i of tile t (same value for all u)
    mall = nc.dram_tensor(
        "mall", [U, B_SHARD], mybir.dt.float32, kind="ExternalInput"
    ).ap()
    iota = nc.dram_tensor("iota", [U, 1], mybir.dt.float32, kind="ExternalInput").ap()
    out = nc.dram_tensor("out", [P, 1], mybir.dt.float32, kind="ExternalOutput").ap()

    with tile.TileContext(nc) as tc:
        with (
            tc.tile_pool(name="sbuf", bufs=4) as pool,
            tc.tile_pool(name="persist", bufs=1) as ppool,
            tc.tile_pool(name="psum", bufs=2, space="PSUM") as psum_pool,
        ):
            uniq_s = ppool.tile([U, N_TILES], mybir.dt.int32)
            nc.gpsimd.dma_start(out=uniq_s[:], in_=uniq)
            mall_s = ppool.tile([U, B_SHARD], mybir.dt.float32)
            nc.gpsimd.dma_start(out=mall_s[:], in_=mall)
            iota_s = ppool.tile([U, 1], mybir.dt.float32)
            nc.gpsimd.dma_start(out=iota_s[:], in_=iota)
            # one-hot selection matrices for every tile: Oall[u, t*128+i]
            oall = ppool.tile([U, B_SHARD], mybir.dt.bfloat16)
            nc.vector.tensor_scalar(
                out=oall[:],
                in0=mall_s[:],
                scalar1=iota_s[:],
                scalar2=None,
                op0=mybir.AluOpType.is_equal,
            )
            acc = ppool.tile([P, N_TILES], mybir.dt.float32)
            for tp in range(N_TILES // 2):
                # two back-to-back 1MB DMAs fill a 2-tile buffer; per-tile
                # landing keeps the subtract/square cadence smooth
                xt2 = pool.tile([P, 2 * FEAT], mybir.dt.float32, tag="xt")
                for half in range(2):
                    t = 2 * tp + half
                    nc.sync.dma_start(
                        out=xt2[:, half * FEAT : (half + 1) * FEAT],
                        in_=x[t * P : (t + 1) * P, :],
                    )
                for half in range(2):
                    t = 2 * tp + half
                    xv = xt2[:, half * FEAT : (half + 1) * FEAT]
                    uhi = pool.tile([U, FEAT], mybir.dt.bfloat16, tag="uhi")
                    nc.gpsimd.indirect_dma_start(
                        out=uhi[:],
                        out_offset=None,
                        in_=chi,
                        in_offset=bass.IndirectOffsetOnAxis(
                            ap=uniq_s[:, t : t + 1], axis=0
                        ),
                    )
                    gp = psum_pool.tile([P, FEAT], mybir.dt.float32, tag="gp")
                    ot = oall[:, t * P : (t + 1) * P]
                    for j in range(NJ):
                        js = slice(j * 512, (j + 1) * 512)
                        nc.tensor.matmul(
                            out=gp[:, js],
                            lhsT=ot,
                            rhs=uhi[:, js],
                            start=True,
                            stop=True,
                        )
                    d = pool.tile([P, FEAT], mybir.dt.float32, tag="d")
                    nc.vector.tensor_tensor(
                        out=d[:], in0=xv, in1=gp[:], op=mybir.AluOpType.subtract
                    )
                    nc.scalar.activation(
                        out=d[:],
                        in_=d[:],
                        func=mybir.ActivationFunctionType.Square,
                        accum_out=acc[:, t : t + 1],
                    )
            accp = ppool.tile([P, 1], mybir.dt.float32)
            nc.vector.tensor_reduce(
                out=accp[:], in_=acc[:], axis=mybir.AxisListType.X, op=mybir.AluOpType.add
            )
            # scalar HWDGE ring is otherwise empty: its completion posts
            # immediately instead of retiring behind the 16 x-stream DMAs
            nc.scalar.dma_start(out=out, in_=accp[:])
    nc.finalize()
    return nc


def _build_v2():
    nc = bacc.Bacc("TRN2", num_devices=N_CORES)
    x = nc.dram_tensor("x", [B_SHARD, FEAT], mybir.dt.float32, kind="ExternalInput").ap()
    labels = nc.dram_tensor(
        "labels", [P, N_TILES], mybir.dt.int32, kind="ExternalInput"
    ).ap()
    cb = nc.dram_tensor(
        "cb", [NUM_CLASSES, FEAT], mybir.dt.bfloat16, kind="ExternalInput"
    ).ap()
    out = nc.dram_tensor("out", [P, 1], mybir.dt.float32, kind="ExternalOutput").ap()

    with tile.TileContext(nc) as tc:
        with (
            tc.tile_pool(name="sbuf", bufs=3) as pool,
            tc.tile_pool(name="persist", bufs=1) as ppool,
        ):
            lab = ppool.tile([P, N_TILES], mybir.dt.int32)
            nc.sync.dma_start(out=lab[:], in_=labels)
            acc = ppool.tile([P, N_TILES], mybir.dt.float32)
            for t in range(N_TILES):
                xt = pool.tile([P, FEAT], mybir.dt.float32, tag="xt")
                nc.sync.dma_start(out=xt[:], in_=x[t * P : (t + 1) * P, :])
                g = pool.tile([P, FEAT], mybir.dt.bfloat16, tag="g")
                nc.gpsimd.indirect_dma_start(
                    out=g[:],
                    out_offset=None,
                    in_=cb,
                    in_offset=bass.IndirectOffsetOnAxis(ap=lab[:, t : t + 1], axis=0),
                )
                d = pool.tile([P, FEAT], mybir.dt.float32, tag="d")
                nc.vector.tensor_tensor(
                    out=d[:], in0=xt[:], in1=g[:], op=mybir.AluOpType.subtract
                )
                nc.scalar.activation(
                    out=d[:],
                    in_=d[:],
                    func=mybir.ActivationFunctionType.Square,
                    accum_out=acc[:, t : t + 1],
                )
            accp = ppool.tile([P, 1], mybir.dt.float32)
            nc.vector.tensor_reduce(
                out=accp[:], in_=acc[:], axis=mybir.AxisListType.X, op=mybir.AluOpType.add
            )
            nc.sync.dma_start(out=out, in_=accp[:])
    nc.finalize()
    return nc


_CACHE = {}


def _find_rotation(labs):
    """Offset o such that every wrapped 2048-row shard of the sorted labels
    touches <= 128 distinct classes (so per-core class sums fit PSUM)."""
    chg = np.concatenate([[1], (labs[1:] != labs[:-1]).astype(np.int64)])
    cum = np.cumsum(chg)

    def ndist(a, b):  # distinct values in labs[a:b], 0 <= a < b <= BATCH
        return int(cum[b - 1] - cum[a]) + 1

    for o in range(B_SHARD):
        ok = True
        for c in range(N_CORES):
            a = c * B_SHARD + o
            b = a + B_SHARD
            if b <= BATCH:
                n = ndist(a, b)
            else:
                n = ndist(a, BATCH) + ndist(0, b - BATCH)
            if n > P:
                ok = False
                break
        if ok:
            return o
    return None


def _prep_v7(x, labels_i, centers):
    order = np.argsort(labels_i, kind="stable")
    o = _find_rotation(labels_i[order])
    if o is None:
        return None
    perm = np.roll(order, -o)
    xdt = ml_dtypes.float8_e3m4 if X_FP8 else ml_dtypes.bfloat16
    cbase = centers.astype(xdt)

    in_maps = []
    for c in range(N_CORES):
        idx = perm[c * B_SHARD : (c + 1) * B_SHARD]
        lc = labels_i[idx]
        uniq, inv = np.unique(lc, return_inverse=True)
        if len(uniq) > P:
            return None
        rows = x[idx].astype(xdt)
        # tile-major relayout: partition p holds row t*128+p of the shard
        xr = np.ascontiguousarray(
            rows.reshape(N_TILES, P, FEAT).transpose(1, 0, 2).reshape(P, N_TILES * FEAT)
        )
        # one-hot lhsT: oh[p, t*128+m] = (local class of row t*128+p == m)
        ohost = np.zeros((P, N_TILES * P), dtype=xdt)
        invr = inv.reshape(N_TILES, P)
        pidx = np.arange(P)
        for t in range(N_TILES):
            ohost[pidx, t * P + invr[t]] = 1.0
        cpad = np.zeros((P, FEAT), dtype=xdt)
        cpad[: len(uniq)] = cbase[uniq]
        cnt = np.zeros((P, 1), dtype=np.float32)
        bc = np.bincount(inv, minlength=P).astype(np.float32)
        cnt[:, 0] = bc[:P]
        in_maps.append({"xq": xr, "oh": ohost, "cb": cpad, "cnt": cnt})
    return in_maps


def _prep_v5(x, labels_i, centers):
    """Sort rows by label, shard, and build per-tile unique/one-hot metadata.

    Returns None if some 128-row tile would need more than U distinct centers.
    """
    order = np.argsort(labels_i, kind="stable")
    labs = labels_i[order]
    chi = centers.astype(ml_dtypes.bfloat16)

    in_maps = []
    for c in range(N_CORES):
        sl = slice(c * B_SHARD, (c + 1) * B_SHARD)
        ls = labs[sl]
        uniq = np.zeros((U, N_TILES), dtype=np.int32)
        mall = np.zeros(B_SHARD, dtype=np.float32)
        for t in range(N_TILES):
            lt = ls[t * P : (t + 1) * P]
            uu, inv = np.unique(lt, return_inverse=True)
            if len(uu) > U:
                return None
            uniq[: len(uu), t] = uu
            uniq[len(uu) :, t] = uu[0]
            mall[t * P : (t + 1) * P] = inv.astype(np.float32)
        in_maps.append(
            {
                "x": np.ascontiguousarray(x[order[sl]]),
                "chi": chi,
                "uniq": uniq,
                "mall": np.ascontiguousarray(
                    np.broadcast_to(mall[None, :], (U, B_SHARD))
                ),
                "iota": np.arange(U, dtype=np.float32).reshape(U, 1),
            }
        )
    return in_maps


def _prep_v2(x, labels_i, centers):
    cb = centers.astype(ml_dtypes.bfloat16)
    in_maps = []
    for c in range(N_CORES):
        sl = slice(c * B_SHARD, (c + 1) * B_SHARD)
        lab = np.ascontiguousarray(
            labels_i[sl].astype(np.int32).reshape(N_TILES, P).T
        )
        in_maps.append({"x": np.ascontiguousarray(x[sl]), "labels": lab, "cb": cb})
    return in_maps


_BUILDERS = {"v7": _build_v7, "v5": _build_v5, "v2": _build_v2}
_PREPPERS = {"v7": _prep_v7, "v5": _prep_v5, "v2": _prep_v2}


def _run(x, labels, centers, trace=False, force=None):
    x = np.ascontiguousarray(np.asarray(x), dtype=np.float32)
    labels_i = np.ascontiguousarray(np.asarray(labels)).astype(np.int64)
    centers = np.ascontiguousarray(np.asarray(centers), dtype=np.float32)
    assert x.shape == (BATCH, FEAT), x.shape
    assert labels_i.shape == (BATCH,), labels_i.shape
    assert centers.shape == (NUM_CLASSES, FEAT), centers.shape

    in_maps = None
    for variant in [force] if force else ["v7", "v5", "v2"]:
        in_maps = _PREPPERS[variant](x, labels_i, centers)
        if in_maps is not None:
            break
    assert in_maps is not None, "no kernel variant applicable"

    if variant not in _CACHE:
        _CACHE[variant] = _BUILDERS[variant]()
    nc = _CACHE[variant]

    res = run_bass_kernel_spmd(nc, in_maps, core_ids=list(range(N_CORES)), trace=trace)
    total = 0.0
    for c in range(N_CORES):
        total += float(res.results[c]["out"].astype(np.float64).sum())
    val = np.float32(total / 2.0 / BATCH)
    return val, res


def kernel(x, labels, centers):
    val, _ = _run(x, labels, centers)
    return val


# revision 17
# speedup vs baseline: 1.0350x; 1.0350x over previous
"""Center loss kernel for Trainium2, 8 NeuronCores, data-parallel over batch.

loss = sum((x - centers[labels])**2) / 2 / BATCH

Primary path (v7): algebraic split
    loss*2*B = sum(x^2) - 2*sum_m S_m.c_m + sum_m count_m*||c_m||^2
where rows are sorted by label on the host (any permutation is valid) and
sharded 8 ways so each core's 2048 rows touch <= 128 distinct classes (a
rotation offset of the sorted order is searched to make that hold), m ranges
over the core's local classes and S = per-class row sums of x.

Per core the device then only needs:
  - stream x as fp8-e3m4 [128, 16*2048] (4.2 MB) on the sync ring, with the
    0.26 MB fp8 one-hot lhsT + first chunk on the parallel scalar ring,
  - 16 one-hot matmuls accumulating S[class, feat] into PSUM (TensorE, the
    critical path: ~216 ns per warm N=512 matmul),
  - sum(x^2) in pair-fused ops split between ScalarE (Square+accum) and
    VectorE (scalar_tensor_tensor (x*1)*x with accum),
  - one tail scalar_tensor_tensor (S * C * -2) and tiny final ops,
  - 0.26 MB center slice + counts late on the scalar HWDGE ring.
HBM traffic per core ~4.8 MB vs 17 MB for the v5 gather scheme and 32 MB for
the naive per-row gather. fp8-e3m4 quantization of x/centers costs ~1.8e-4
rel err (gate is 2e-2); X_FP8=False gives a bf16 stream at ~8e-6 instead.

Fallback paths when the class-packing precondition fails: v5 (sorted tiles,
<=16 distinct centers per 128-row tile) then v2 (any label distribution).

Each core returns 128x3 partial sums; the host adds them in f64 and applies
the 1/(2*BATCH) scale.
"""

import numpy as np
import ml_dtypes

import concourse.bacc as bacc
import concourse.bass as bass
import concourse.mybir as mybir
import concourse.tile as tile
from concourse.bass_utils import run_bass_kernel_spmd

N_CORES = 8
BATCH = 16384
FEAT = 2048
NUM_CLASSES = 1000
B_SHARD = BATCH // N_CORES  # 2048
P = 128
N_TILES = B_SHARD // P  # 16
U = 16  # max distinct centers per 128-row tile (sorted labels, v5)
NJ = FEAT // 512  # matmul free-dim chunks
NCHUNKS = 8  # x-stream DMA chunks (v7)


X_FP8 = True  # stream x/oh/cb as fp8-e3m4 (rel err ~1.5e-4) vs bf16 (~2e-6)


def _build_v7():
    xdt = mybir.dt.float8e3 if X_FP8 else mybir.dt.bfloat16
    nc = bacc.Bacc("TRN2", num_devices=N_CORES)
    xq = nc.dram_tensor("xq", [P, N_TILES * FEAT], xdt, kind="ExternalInput").ap()
    oh = nc.dram_tensor("oh", [P, N_TILES * P], xdt, kind="ExternalInput").ap()
    cb = nc.dram_tensor("cb", [P, FEAT], xdt, kind="ExternalInput").ap()
    cnt = nc.dram_tensor("cnt", [P, 1], mybir.dt.float32, kind="ExternalInput").ap()
    out = nc.dram_tensor("out", [P, 3], mybir.dt.float32, kind="ExternalOutput").ap()

    # x-stream chunking (in tiles): small first chunks so compute starts
    # earlier. Everything the hot path needs (oh, then tile 0 first) rides
    # the sync ring, whose first descriptor moves ~1us before the scalar
    # ring's (the scalar ring sits behind the ACT table load). cb/cnt are
    # only needed late and go on the scalar ring mid-kernel.
    chunk_tiles = [1, 1, 2, 2, 2, 2, 2, 2, 2]
    assert sum(chunk_tiles) == N_TILES
    # squares: pair-fused instructions amortize per-op overhead; singles for
    # the two ramp tiles so ACT starts on tile 0 and DVE on tile 1.
    # (engine, start_tile, ntiles)
    squares = [
        ("a", 0, 1),
        ("v", 1, 1),
        ("a", 2, 2),
        ("v", 4, 2),
        ("a", 6, 2),
        ("a", 8, 2),
        ("v", 10, 2),
        ("a", 12, 2),
        ("v", 14, 2),
    ]

    with tile.TileContext(nc) as tc:
        with (
            tc.tile_pool(name="persist", bufs=1) as ppool,
            tc.tile_pool(name="psum", bufs=1, space="PSUM") as psum_pool,
        ):
            oh_s = ppool.tile([P, N_TILES * P], xdt)
            nc.sync.dma_start(out=oh_s[:], in_=oh)

            xbuf = ppool.tile([P, N_TILES * FEAT], xdt)
            t0 = 0
            for ntk in chunk_tiles:
                nc.sync.dma_start(
                    out=xbuf[:, t0 * FEAT : (t0 + ntk) * FEAT],
                    in_=xq[:, t0 * FEAT : (t0 + ntk) * FEAT],
                )
                t0 += ntk
            cb_s = ppool.tile([P, FEAT], xdt)
            nc.scalar.dma_start(out=cb_s[:], in_=cb)
            cnt_s = ppool.tile([P, 1], mybir.dt.float32)
            nc.scalar.dma_start(out=cnt_s[:], in_=cnt)

            S = psum_pool.tile([P, FEAT], mybir.dt.float32)
            qacc = ppool.tile([P, N_TILES], mybir.dt.float32)
            nc.vector.memset(qacc[:], 0.0)
            sa = ppool.tile([P, 2 * FEAT], mybir.dt.bfloat16)  # ACT scratch
            sv = ppool.tile([P, 2 * FEAT], mybir.dt.bfloat16)  # DVE scratch

            sq = {t: (eng, nt) for eng, t, nt in squares}
            for t in range(N_TILES):
                ot = oh_s[:, t * P : (t + 1) * P]
                for j in range(NJ):
                    js = slice(j * 512, (j + 1) * 512)
                    nc.tensor.matmul(
                        out=S[:, js],
                        lhsT=ot,
                        rhs=xbuf[:, t * FEAT + j * 512 : t * FEAT + (j + 1) * 512],
                        start=(t == 0),
                        stop=(t == N_TILES - 1),
                    )
                if t not in sq:
                    continue
                eng, nt = sq[t]
                xt = xbuf[:, t * FEAT : (t + nt) * FEAT]
                if eng == "a":
                    nc.scalar.activation(
                        out=sa[:, : nt * FEAT],
                        in_=xt,
                        func=mybir.ActivationFunctionType.Square,
                        accum_out=qacc[:, t : t + 1],
                    )
                else:
                    # fused square+row-sum: out=(x*1)*x, accum_out=sum(out)
                    # (native TENSOR_TENSOR_REDUCE crashes the exec unit on
                    # this runtime; InstTensorScalarPtr works)
                    nc.vector.scalar_tensor_tensor(
                        out=sv[:, : nt * FEAT],
                        in0=xt,
                        scalar=1.0,
                        in1=xt,
                        op0=mybir.AluOpType.mult,
                        op1=mybir.AluOpType.mult,
                        accum_out=qacc[:, t : t + 1],
                    )
            partials = ppool.tile([P, 3], mybir.dt.float32)
            # cc term fused into one DVE op: sum_n (c*count)*c = count*||c||^2
            psc = ppool.tile([P, FEAT], mybir.dt.bfloat16)
            nc.vector.scalar_tensor_tensor(
                out=psc[:],
                in0=cb_s[:],
                scalar=cnt_s[:],
                in1=cb_s[:],
                op0=mybir.AluOpType.mult,
                op1=mybir.AluOpType.mult,
                accum_out=partials[:, 2:3],
            )
            nc.vector.tensor_reduce(
                out=partials[:, 0:1],
                in_=qacc[:],
                axis=mybir.AxisListType.X,
                op=mybir.AluOpType.add,
            )
            # tail: cross term -2 * sum_n S[m,n]*c[m,n]
            nc.vector.scalar_tensor_tensor(
                out=psc[:],
                in0=S[:],
                scalar=-2.0,
                in1=cb_s[:],
                op0=mybir.AluOpType.mult,
                op1=mybir.AluOpType.mult,
                accum_out=partials[:, 1:2],
            )
            nc.scalar.dma_start(out=out, in_=partials[:])
    nc.finalize()
    return nc


def _build_v5():
    nc = bacc.Bacc("TRN2", num_devices=N_CORES)
    x = nc.dram_tensor("x", [B_SHARD, FEAT], mybir.dt.float32, kind="ExternalInput").ap()
    chi = nc.dram_tensor(
        "chi", [NUM_CLASSES, FEAT], mybir.dt.bfloat16, kind="ExternalInput"
    ).ap()
    uniq = nc.dram_tensor("uniq", [U, N_TILES], mybir.dt.int32, kind="ExternalInput").ap()
    # mall[u, t*128 + i] = one-hot slot of row i of tile t (same value for all u)
    mall = nc.dram_tensor(
        "mall", [U, B_SHARD], mybir.dt.float32, kind="ExternalInput"
    ).ap()
    iota = nc.dram_tensor("iota", [U, 1], mybir.dt.float32, kind="ExternalInput").ap()
    out = nc.dram_tensor("out", [P, 1], mybir.dt.float32, kind="ExternalOutput").ap()

    with tile.TileContext(nc) as tc:
        with (
            tc.tile_pool(name="sbuf", bufs=4) as pool,
            tc.tile_pool(name="persist", bufs=1) as ppool,
            tc.tile_pool(name="psum", bufs=2, space="PSUM") as psum_pool,
        ):
            uniq_s = ppool.tile([U, N_TILES], mybir.dt.int32)
            nc.gpsimd.dma_start(out=uniq_s[:], in_=uniq)
            mall_s = ppool.tile([U, B_SHARD], mybir.dt.float32)
            nc.gpsimd.dma_start(out=mall_s[:], in_=mall)
            iota_s = ppool.tile([U, 1], mybir.dt.float32)
            nc.gpsimd.dma_start(out=iota_s[:], in_=iota)
            # one-hot selection matrices for every tile: Oall[u, t*128+i]
            oall = ppool.tile([U, B_SHARD], mybir.dt.bfloat16)
            nc.vector.tensor_scalar(
                out=oall[:],
                in0=mall_s[:],
                scalar1=iota_s[:],
                scalar2=None,
                op0=mybir.AluOpType.is_equal,
            )
            acc = ppool.tile([P, N_TILES], mybir.dt.float32)
            for tp in range(N_TILES // 2):
                # two back-to-back 1MB DMAs fill a 2-tile buffer; per-tile
                # landing keeps the subtract/square cadence smooth
                xt2 = pool.tile([P, 2 * FEAT], mybir.dt.float32, tag="xt")
                for half in range(2):
                    t = 2 * tp + half
                    nc.sync.dma_start(
                        out=xt2[:, half * FEAT : (half + 1) * FEAT],
                        in_=x[t * P : (t + 1) * P, :],
                    )
                for half in range(2):
                    t = 2 * tp + half
                    xv = xt2[:, half * FEAT : (half + 1) * FEAT]
                    uhi = pool.tile([U, FEAT], mybir.dt.bfloat16, tag="uhi")
                    nc.gpsimd.indirect_dma_start(
                        out=uhi[:],
                        out_offset=None,
                        in_=chi,
                        in_offset=bass.IndirectOffsetOnAxis(
                            ap=uniq_s[:, t : t + 1], axis=0
                        ),
                    )
                    gp = psum_pool.tile([P, FEAT], mybir.dt.float32, tag="gp")
                    ot = oall[:, t * P : (t + 1) * P]
                    for j in range(NJ):
                        js = slice(j * 512, (j + 1) * 512)
                        nc.tensor.matmul(
                            out=gp[:, js],
                            lhsT=ot,
                            rhs=uhi[:, js],
                            start=True,
                            stop=True,
                        )
                    d = pool.tile([P, FEAT], mybir.dt.float32, tag="d")
                    nc.vector.tensor_tensor(
                        out=d[:], in0=xv, in1=gp[:], op=mybir.AluOpType.subtract
                    )
                    nc.scalar.activation(
                        out=d[:],
                        in_=d[:],
                        func=mybir.ActivationFunctionType.Square,
                        accum_out=acc[:, t : t + 1],
                    )
            accp = ppool.tile([P, 1], mybir.dt.float32)
            nc.vector.tensor_reduce(
                out=accp[:], in_=acc[:], axis=mybir.AxisListType.X, op=mybir.AluOpType.add
            )
            # scalar HWDGE ring is otherwise empty: its completion posts
            # immediately instead of retiring behind the 16 x-stream DMAs
            nc.scalar.dma_start(out=out, in_=accp[:])
    nc.finalize()
    return nc


def _build_v2():
    nc = bacc.Bacc("TRN2", num_devices=N_CORES)
    x = nc.dram_tensor("x", [B_SHARD, FEAT], mybir.dt.float32, kind="ExternalInput").ap()
    labels = nc.dram_tensor(
        "labels", [P, N_TILES], mybir.dt.int32, kind="ExternalInput"
    ).ap()
    cb = nc.dram_tensor(
        "cb", [NUM_CLASSES, FEAT], mybir.dt.bfloat16, kind="ExternalInput"
    ).ap()
    out = nc.dram_tensor("out", [P, 1], mybir.dt.float32, kind="ExternalOutput").ap()

    with tile.TileContext(nc) as tc:
        with (
            tc.tile_pool(name="sbuf", bufs=3) as pool,
            tc.tile_pool(name="persist", bufs=1) as ppool,
        ):
            lab = ppool.tile([P, N_TILES], mybir.dt.int32)
            nc.sync.dma_start(out=lab[:], in_=labels)
            acc = ppool.tile([P, N_TILES], mybir.dt.float32)
            for t in range(N_TILES):
                xt = pool.tile([P, FEAT], mybir.dt.float32, tag="xt")
                nc.sync.dma_start(out=xt[:], in_=x[t * P : (t + 1) * P, :])
                g = pool.tile([P, FEAT], mybir.dt.bfloat16, tag="g")
                nc.gpsimd.indirect_dma_start(
                    out=g[:],
                    out_offset=None,
                    in_=cb,
                    in_offset=bass.IndirectOffsetOnAxis(ap=lab[:, t : t + 1], axis=0),
                )
                d = pool.tile([P, FEAT], mybir.dt.float32, tag="d")
                nc.vector.tensor_tensor(
                    out=d[:], in0=xt[:], in1=g[:], op=mybir.AluOpType.subtract
                )
                nc.scalar.activation(
                    out=d[:],
                    in_=d[:],
                    func=mybir.ActivationFunctionType.Square,
                    accum_out=acc[:, t : t + 1],
                )
            accp = ppool.tile([P, 1], mybir.dt.float32)
            nc.vector.tensor_reduce(
                out=accp[:], in_=acc[:], axis=mybir.AxisListType.X, op=mybir.AluOpType.add
            )
            nc.sync.dma_start(out=out, in_=accp[:])
    nc.finalize()
    return nc


_CACHE = {}


def _find_rotation(labs):
    """Offset o such that every wrapped 2048-row shard of the sorted labels
    touches <= 128 distinct classes (so per-core class sums fit PSUM)."""
    chg = np.concatenate([[1], (labs[1:] != labs[:-1]).astype(np.int64)])
    cum = np.cumsum(chg)

    def ndist(a, b):  # distinct values in labs[a:b], 0 <= a < b <= BATCH
        return int(cum[b - 1] - cum[a]) + 1

    for o in range(B_SHARD):
        ok = True
        for c in range(N_CORES):
            a = c * B_SHARD + o
            b = a + B_SHARD
            if b <= BATCH:
                n = ndist(a, b)
            else:
                n = ndist(a, BATCH) + ndist(0, b - BATCH)
            if n > P:
                ok = False
                break
        if ok:
            return o
    return None


def _prep_v7(x, labels_i, centers):
    order = np.argsort(labels_i, kind="stable")
    o = _find_rotation(labels_i[order])
    if o is None:
        return None
    perm = np.roll(order, -o)
    xdt = ml_dtypes.float8_e3m4 if X_FP8 else ml_dtypes.bfloat16
    cbase = centers.astype(xdt)

    in_maps = []
    for c in range(N_CORES):
        idx = perm[c * B_SHARD : (c + 1) * B_SHARD]
        lc = labels_i[idx]
        uniq, inv = np.unique(lc, return_inverse=True)
        if len(uniq) > P:
            return None
        rows = x[idx].astype(xdt)
        # tile-major relayout: partition p holds row t*128+p of the shard
        xr = np.ascontiguousarray(
            rows.reshape(N_TILES, P, FEAT).transpose(1, 0, 2).reshape(P, N_TILES * FEAT)
        )
        # one-hot lhsT: oh[p, t*128+m] = (local class of row t*128+p == m)
        ohost = np.zeros((P, N_TILES * P), dtype=xdt)
        invr = inv.reshape(N_TILES, P)
        pidx = np.arange(P)
        for t in range(N_TILES):
            ohost[pidx, t * P + invr[t]] = 1.0
        cpad = np.zeros((P, FEAT), dtype=xdt)
        cpad[: len(uniq)] = cbase[uniq]
        cnt = np.zeros((P, 1), dtype=np.float32)
        bc = np.bincount(inv, minlength=P).astype(np.float32)
        cnt[:, 0] = bc[:P]
        in_maps.append({"xq": xr, "oh": ohost, "cb": cpad, "cnt": cnt})
    return in_maps


def _prep_v5(x, labels_i, centers):
    """Sort rows by label, shard, and build per-tile unique/one-hot metadata.

    Returns None if some 128-row tile would need more than U distinct centers.
    """
    order = np.argsort(labels_i, kind="stable")
    labs = labels_i[order]
    chi = centers.astype(ml_dtypes.bfloat16)

    in_maps = []
    for c in range(N_CORES):
        sl = slice(c * B_SHARD, (c + 1) * B_SHARD)
        ls = labs[sl]
        uniq = np.zeros((U, N_TILES), dtype=np.int32)
        mall = np.zeros(B_SHARD, dtype=np.float32)
        for t in range(N_TILES):
            lt = ls[t * P : (t + 1) * P]
            uu, inv = np.unique(lt, return_inverse=True)
            if len(uu) > U:
                return None
            uniq[: len(uu), t] = uu
            uniq[len(uu) :, t] = uu[0]
            mall[t * P : (t + 1) * P] = inv.astype(np.float32)
        in_maps.append(
            {
                "x": np.ascontiguousarray(x[order[sl]]),
                "chi": chi,
                "uniq": uniq,
                "mall": np.ascontiguousarray(
                    np.broadcast_to(mall[None, :], (U, B_SHARD))
                ),
                "iota": np.arange(U, dtype=np.float32).reshape(U, 1),
            }
        )
    return in_maps


def _prep_v2(x, labels_i, centers):
    cb = centers.astype(ml_dtypes.bfloat16)
    in_maps = []
    for c in range(N_CORES):
        sl = slice(c * B_SHARD, (c + 1) * B_SHARD)
        lab = np.ascontiguousarray(
            labels_i[sl].astype(np.int32).reshape(N_TILES, P).T
        )
        in_maps.append({"x": np.ascontiguousarray(x[sl]), "labels": lab, "cb": cb})
    return in_maps


_BUILDERS = {"v7": _build_v7, "v5": _build_v5, "v2": _build_v2}
_PREPPERS = {"v7": _prep_v7, "v5": _prep_v5, "v2": _prep_v2}


def _run(x, labels, centers, trace=False, force=None):
    x = np.ascontiguousarray(np.asarray(x), dtype=np.float32)
    labels_i = np.ascontiguousarray(np.asarray(labels)).astype(np.int64)
    centers = np.ascontiguousarray(np.asarray(centers), dtype=np.float32)
    assert x.shape == (BATCH, FEAT), x.shape
    assert labels_i.shape == (BATCH,), labels_i.shape
    assert centers.shape == (NUM_CLASSES, FEAT), centers.shape

    in_maps = None
    for variant in [force] if force else ["v7", "v5", "v2"]:
        in_maps = _PREPPERS[variant](x, labels_i, centers)
        if in_maps is not None:
            break
    assert in_maps is not None, "no kernel variant applicable"

    if variant not in _CACHE:
        _CACHE[variant] = _BUILDERS[variant]()
    nc = _CACHE[variant]

    res = run_bass_kernel_spmd(nc, in_maps, core_ids=list(range(N_CORES)), trace=trace)
    total = 0.0
    for c in range(N_CORES):
        total += float(res.results[c]["out"].astype(np.float64).sum())
    val = np.float32(total / 2.0 / BATCH)
    return val, res


def kernel(x, labels, centers):
    val, _ = _run(x, labels, centers)
    return val


# revision 19
# speedup vs baseline: 1.0631x; 1.0271x over previous
"""Center loss kernel for Trainium2, 8 NeuronCores, data-parallel over batch.

loss = sum((x - centers[labels])**2) / 2 / BATCH

Primary path (v7): algebraic split
    loss*2*B = sum(x^2) - 2*sum_m S_m.c_m + sum_m count_m*||c_m||^2
where rows are sorted by label on the host (any permutation is valid) and
sharded 8 ways so each core's 2048 rows touch <= 128 distinct classes (a
rotation offset of the sorted order is searched to make that hold), m ranges
over the core's local classes and S = per-class row sums of x.

Per core the device then only needs:
  - stream x as fp8-e3m4 [128, 16*2048] (4.2 MB) on the sync ring, with the
    0.26 MB fp8 one-hot lhsT + first chunk on the parallel scalar ring,
  - 16 one-hot matmuls accumulating S[class, feat] into PSUM (TensorE, the
    critical path: ~216 ns per warm N=512 matmul),
  - sum(x^2) in pair-fused ops split between ScalarE (Square+accum) and
    VectorE (scalar_tensor_tensor (x*1)*x with accum),
  - tail scalar_tensor_tensor ops: (S * -2) * C for the cross term and
    (C * count) * C for the center-norm term, plus one tensor_reduce,
  - 0.26 MB center slice + counts late on the scalar HWDGE ring.
HBM traffic per core ~4.8 MB vs 17 MB for the v5 gather scheme and 32 MB for
the naive per-row gather. fp8-e3m4 quantization of x/centers costs ~1.8e-4
rel err (gate is 2e-2); X_FP8=False gives a bf16 stream at ~8e-6 instead.

Fallback paths when the class-packing precondition fails: v5 (sorted tiles,
<=16 distinct centers per 128-row tile) then v2 (any label distribution).

Each core returns 128x3 partial sums; the host adds them in f64 and applies
the 1/(2*BATCH) scale.
"""

import numpy as np
import ml_dtypes

import concourse.bacc as bacc
import concourse.bass as bass
import concourse.mybir as mybir
import concourse.tile as tile
from concourse.bass_utils import run_bass_kernel_spmd

N_CORES = 8
BATCH = 16384
FEAT = 2048
NUM_CLASSES = 1000
B_SHARD = BATCH // N_CORES  # 2048
P = 128
N_TILES = B_SHARD // P  # 16
U = 16  # max distinct centers per 128-row tile (sorted labels, v5)
NJ = FEAT // 512  # matmul free-dim chunks
NCHUNKS = 8  # x-stream DMA chunks (v7)


X_FP8 = True  # stream x/oh/cb as fp8-e3m4 (rel err ~1.5e-4) vs bf16 (~2e-6)


def _build_v7():
    xdt = mybir.dt.float8e3 if X_FP8 else mybir.dt.bfloat16
    nc = bacc.Bacc("TRN2", num_devices=N_CORES)
    xq = nc.dram_tensor("xq", [P, N_TILES * FEAT], xdt, kind="ExternalInput").ap()
    oh = nc.dram_tensor("oh", [P, N_TILES * P], xdt, kind="ExternalInput").ap()
    cb = nc.dram_tensor("cb", [P, FEAT], xdt, kind="ExternalInput").ap()
    cnt = nc.dram_tensor("cnt", [P, 1], mybir.dt.float32, kind="ExternalInput").ap()
    out = nc.dram_tensor("out", [P, 3], mybir.dt.float32, kind="ExternalOutput").ap()

    # x-stream chunking (in tiles): small first chunks so compute starts
    # earlier. Everything the hot path needs (oh, then tile 0 first) rides
    # the sync ring, whose first descriptor moves ~1us before the scalar
    # ring's (the scalar ring sits behind the ACT table load). cb/cnt are
    # only needed late and go on the scalar ring mid-kernel.
    chunk_tiles = [1, 1, 2, 2, 2, 2, 2, 2, 2]
    assert sum(chunk_tiles) == N_TILES
    # squares: pair-fused instructions amortize per-op overhead; singles for
    # the two ramp tiles so ACT starts on tile 0 and DVE on tile 1.
    # (engine, start_tile, ntiles)
    squares = [
        ("a", 0, 1),
        ("v", 1, 1),
        ("a", 2, 2),
        ("v", 4, 2),
        ("a", 6, 2),
        ("a", 8, 2),
        ("v", 10, 2),
        ("a", 12, 2),
        ("v", 14, 2),
    ]

    with tile.TileContext(nc) as tc:
        with (
            tc.tile_pool(name="persist", bufs=1) as ppool,
            tc.tile_pool(name="psum", bufs=1, space="PSUM") as psum_pool,
        ):
            # ring order tile0, oh, tile1, ...: ACT's first square only needs
            # tile 0, so it starts one transfer earlier; PE needs oh anyway
            oh_s = ppool.tile([P, N_TILES * P], xdt)
            xbuf = ppool.tile([P, N_TILES * FEAT], xdt)
            t0 = 0
            for k, ntk in enumerate(chunk_tiles):
                nc.sync.dma_start(
                    out=xbuf[:, t0 * FEAT : (t0 + ntk) * FEAT],
                    in_=xq[:, t0 * FEAT : (t0 + ntk) * FEAT],
                )
                t0 += ntk
                if k == 0:
                    nc.sync.dma_start(out=oh_s[:], in_=oh)
            cb_s = ppool.tile([P, FEAT], xdt)
            nc.scalar.dma_start(out=cb_s[:], in_=cb)
            cnt_s = ppool.tile([P, 1], mybir.dt.float32)
            nc.scalar.dma_start(out=cnt_s[:], in_=cnt)

            S = psum_pool.tile([P, FEAT], mybir.dt.float32)
            qacc = ppool.tile([P, N_TILES], mybir.dt.float32)
            nc.vector.memset(qacc[:], 0.0)
            sa = ppool.tile([P, 2 * FEAT], mybir.dt.bfloat16)  # ACT scratch
            sv = ppool.tile([P, 2 * FEAT], mybir.dt.bfloat16)  # DVE scratch

            sq = {t: (eng, nt) for eng, t, nt in squares}
            for t in range(N_TILES):
                ot = oh_s[:, t * P : (t + 1) * P]
                for j in range(NJ):
                    js = slice(j * 512, (j + 1) * 512)
                    nc.tensor.matmul(
                        out=S[:, js],
                        lhsT=ot,
                        rhs=xbuf[:, t * FEAT + j * 512 : t * FEAT + (j + 1) * 512],
                        start=(t == 0),
                        stop=(t == N_TILES - 1),
                    )
                if t not in sq:
                    continue
                eng, nt = sq[t]
                xt = xbuf[:, t * FEAT : (t + nt) * FEAT]
                if eng == "a":
                    nc.scalar.activation(
                        out=sa[:, : nt * FEAT],
                        in_=xt,
                        func=mybir.ActivationFunctionType.Square,
                        accum_out=qacc[:, t : t + 1],
                    )
                else:
                    # fused square+row-sum: out=(x*1)*x, accum_out=sum(out)
                    # (native TENSOR_TENSOR_REDUCE crashes the exec unit on
                    # this runtime; InstTensorScalarPtr works)
                    nc.vector.scalar_tensor_tensor(
                        out=sv[:, : nt * FEAT],
                        in0=xt,
                        scalar=1.0,
                        in1=xt,
                        op0=mybir.AluOpType.mult,
                        op1=mybir.AluOpType.mult,
                        accum_out=qacc[:, t : t + 1],
                    )
            partials = ppool.tile([P, 3], mybir.dt.float32)
            # cc term fused into one DVE op: sum_n (c*count)*c = count*||c||^2
            psc = ppool.tile([P, FEAT], mybir.dt.bfloat16)
            nc.vector.scalar_tensor_tensor(
                out=psc[:],
                in0=cb_s[:],
                scalar=cnt_s[:],
                in1=cb_s[:],
                op0=mybir.AluOpType.mult,
                op1=mybir.AluOpType.mult,
                accum_out=partials[:, 2:3],
            )
            nc.vector.tensor_reduce(
                out=partials[:, 0:1],
                in_=qacc[:],
                axis=mybir.AxisListType.X,
                op=mybir.AluOpType.add,
            )
            # tail: cross term -2 * sum_n S[m,n]*c[m,n]
            nc.vector.scalar_tensor_tensor(
                out=psc[:],
                in0=S[:],
                scalar=-2.0,
                in1=cb_s[:],
                op0=mybir.AluOpType.mult,
                op1=mybir.AluOpType.mult,
                accum_out=partials[:, 1:2],
            )
            nc.scalar.dma_start(out=out, in_=partials[:])
    nc.finalize()
    return nc


def _build_v5():
    nc = bacc.Bacc("TRN2", num_devices=N_CORES)
    x = nc.dram_tensor("x", [B_SHARD, FEAT], mybir.dt.float32, kind="ExternalInput").ap()
    chi = nc.dram_tensor(
        "chi", [NUM_CLASSES, FEAT], mybir.dt.bfloat16, kind="ExternalInput"
    ).ap()
    uniq = nc.dram_tensor("uniq", [U, N_TILES], mybir.dt.int32, kind="ExternalInput").ap()
    # mall[u, t*128 + i] = one-hot slot of row i of tile t (same value for all u)
    mall = nc.dram_tensor(
        "mall", [U, B_SHARD], mybir.dt.float32, kind="ExternalInput"
    ).ap()
    iota = nc.dram_tensor("iota", [U, 1], mybir.dt.float32, kind="ExternalInput").ap()
    out = nc.dram_tensor("out", [P, 1], mybir.dt.float32, kind="ExternalOutput").ap()

    with tile.TileContext(nc) as tc:
        with (
            tc.tile_pool(name="sbuf", bufs=4) as pool,
            tc.tile_pool(name="persist", bufs=1) as ppool,
            tc.tile_pool(name="psum", bufs=2, space="PSUM") as psum_pool,
        ):
            uniq_s = ppool.tile([U, N_TILES], mybir.dt.int32)
            nc.gpsimd.dma_start(out=uniq_s[:], in_=uniq)
            mall_s = ppool.tile([U, B_SHARD], mybir.dt.float32)
            nc.gpsimd.dma_start(out=mall_s[:], in_=mall)
            iota_s = ppool.tile([U, 1], mybir.dt.float32)
            nc.gpsimd.dma_start(out=iota_s[:], in_=iota)
            # one-hot selection matrices for every tile: Oall[u, t*128+i]
            oall = ppool.tile([U, B_SHARD], mybir.dt.bfloat16)
            nc.vector.tensor_scalar(
                out=oall[:],
                in0=mall_s[:],
                scalar1=iota_s[:],
                scalar2=None,
                op0=mybir.AluOpType.is_equal,
            )
            acc = ppool.tile([P, N_TILES], mybir.dt.float32)
            for tp in range(N_TILES // 2):
                # two back-to-back 1MB DMAs fill a 2-tile buffer; per-tile
                # landing keeps the subtract/square cadence smooth
                xt2 = pool.tile([P, 2 * FEAT], mybir.dt.float32, tag="xt")
                for half in range(2):
                    t = 2 * tp + half
                    nc.sync.dma_start(
                        out=xt2[:, half * FEAT : (half + 1) * FEAT],
                        in_=x[t * P : (t + 1) * P, :],
                    )
                for half in range(2):
                    t = 2 * tp + half
                    xv = xt2[:, half * FEAT : (half + 1) * FEAT]
                    uhi = pool.tile([U, FEAT], mybir.dt.bfloat16, tag="uhi")
                    nc.gpsimd.indirect_dma_start(
                        out=uhi[:],
                        out_offset=None,
                        in_=chi,
                        in_offset=bass.IndirectOffsetOnAxis(
                            ap=uniq_s[:, t : t + 1], axis=0
                        ),
                    )
                    gp = psum_pool.tile([P, FEAT], mybir.dt.float32, tag="gp")
                    ot = oall[:, t * P : (t + 1) * P]
                    for j in range(NJ):
                        js = slice(j * 512, (j + 1) * 512)
                        nc.tensor.matmul(
                            out=gp[:, js],
                            lhsT=ot,
                            rhs=uhi[:, js],
                            start=True,
                            stop=True,
                        )
                    d = pool.tile([P, FEAT], mybir.dt.float32, tag="d")
                    nc.vector.tensor_tensor(
                        out=d[:], in0=xv, in1=gp[:], op=mybir.AluOpType.subtract
                    )
                    nc.scalar.activation(
                        out=d[:],
                        in_=d[:],
                        func=mybir.ActivationFunctionType.Square,
                        accum_out=acc[:, t : t + 1],
                    )
            accp = ppool.tile([P, 1], mybir.dt.float32)
            nc.vector.tensor_reduce(
                out=accp[:], in_=acc[:], axis=mybir.AxisListType.X, op=mybir.AluOpType.add
            )
            # scalar HWDGE ring is otherwise empty: its completion posts
            # immediately instead of retiring behind the 16 x-stream DMAs
            nc.scalar.dma_start(out=out, in_=accp[:])
    nc.finalize()
    return nc


def _build_v2():
    nc = bacc.Bacc("TRN2", num_devices=N_CORES)
    x = nc.dram_tensor("x", [B_SHARD, FEAT], mybir.dt.float32, kind="ExternalInput").ap()
    labels = nc.dram_tensor(
        "labels", [P, N_TILES], mybir.dt.int32, kind="ExternalInput"
    ).ap()
    cb = nc.dram_tensor(
        "cb", [NUM_CLASSES, FEAT], mybir.dt.bfloat16, kind="ExternalInput"
    ).ap()
    out = nc.dram_tensor("out", [P, 1], mybir.dt.float32, kind="ExternalOutput").ap()

    with tile.TileContext(nc) as tc:
        with (
            tc.tile_pool(name="sbuf", bufs=3) as pool,
            tc.tile_pool(name="persist", bufs=1) as ppool,
        ):
            lab = ppool.tile([P, N_TILES], mybir.dt.int32)
            nc.sync.dma_start(out=lab[:], in_=labels)
            acc = ppool.tile([P, N_TILES], mybir.dt.float32)
            for t in range(N_TILES):
                xt = pool.tile([P, FEAT], mybir.dt.float32, tag="xt")
                nc.sync.dma_start(out=xt[:], in_=x[t * P : (t + 1) * P, :])
                g = pool.tile([P, FEAT], mybir.dt.bfloat16, tag="g")
                nc.gpsimd.indirect_dma_start(
                    out=g[:],
                    out_offset=None,
                    in_=cb,
                    in_offset=bass.IndirectOffsetOnAxis(ap=lab[:, t : t + 1], axis=0),
                )
                d = pool.tile([P, FEAT], mybir.dt.float32, tag="d")
                nc.vector.tensor_tensor(
                    out=d[:], in0=xt[:], in1=g[:], op=mybir.AluOpType.subtract
                )
                nc.scalar.activation(
                    out=d[:],
                    in_=d[:],
                    func=mybir.ActivationFunctionType.Square,
                    accum_out=acc[:, t : t + 1],
                )
            accp = ppool.tile([P, 1], mybir.dt.float32)
            nc.vector.tensor_reduce(
                out=accp[:], in_=acc[:], axis=mybir.AxisListType.X, op=mybir.AluOpType.add
            )
            nc.sync.dma_start(out=out, in_=accp[:])
    nc.finalize()
    return nc


_CACHE = {}


def _find_rotation(labs):
    """Offset o such that every wrapped 2048-row shard of the sorted labels
    touches <= 128 distinct classes (so per-core class sums fit PSUM)."""
    chg = np.concatenate([[1], (labs[1:] != labs[:-1]).astype(np.int64)])
    cum = np.cumsum(chg)

    def ndist(a, b):  # distinct values in labs[a:b], 0 <= a < b <= BATCH
        return int(cum[b - 1] - cum[a]) + 1

    for o in range(B_SHARD):
        ok = True
        for c in range(N_CORES):
            a = c * B_SHARD + o
            b = a + B_SHARD
            if b <= BATCH:
                n = ndist(a, b)
            else:
                n = ndist(a, BATCH) + ndist(0, b - BATCH)
            if n > P:
                ok = False
                break
        if ok:
            return o
    return None


def _prep_v7(x, labels_i, centers):
    order = np.argsort(labels_i, kind="stable")
    o = _find_rotation(labels_i[order])
    if o is None:
        return None
    perm = np.roll(order, -o)
    xdt = ml_dtypes.float8_e3m4 if X_FP8 else ml_dtypes.bfloat16
    cbase = centers.astype(xdt)

    in_maps = []
    for c in range(N_CORES):
        idx = perm[c * B_SHARD : (c + 1) * B_SHARD]
        lc = labels_i[idx]
        uniq, inv = np.unique(lc, return_inverse=True)
        if len(uniq) > P:
            return None
        rows = x[idx].astype(xdt)
        # tile-major relayout: partition p holds row t*128+p of the shard
        xr = np.ascontiguousarray(
            rows.reshape(N_TILES, P, FEAT).transpose(1, 0, 2).reshape(P, N_TILES * FEAT)
        )
        # one-hot lhsT: oh[p, t*128+m] = (local class of row t*128+p == m)
        ohost = np.zeros((P, N_TILES * P), dtype=xdt)
        invr = inv.reshape(N_TILES, P)
        pidx = np.arange(P)
        for t in range(N_TILES):
            ohost[pidx, t * P + invr[t]] = 1.0
        cpad = np.zeros((P, FEAT), dtype=xdt)
        cpad[: len(uniq)] = cbase[uniq]
        cnt = np.zeros((P, 1), dtype=np.float32)
        bc = np.bincount(inv, minlength=P).astype(np.float32)
        cnt[:, 0] = bc[:P]
        in_maps.append({"xq": xr, "oh": ohost, "cb": cpad, "cnt": cnt})
    return in_maps


def _prep_v5(x, labels_i, centers):
    """Sort rows by label, shard, and build per-tile unique/one-hot metadata.

    Returns None if some 128-row tile would need more than U distinct centers.
    """
    order = np.argsort(labels_i, kind="stable")
    labs = labels_i[order]
    chi = centers.astype(ml_dtypes.bfloat16)

    in_maps = []
    for c in range(N_CORES):
        sl = slice(c * B_SHARD, (c + 1) * B_SHARD)
        ls = labs[sl]
        uniq = np.zeros((U, N_TILES), dtype=np.int32)
        mall = np.zeros(B_SHARD, dtype=np.float32)
        for t in range(N_TILES):
            lt = ls[t * P : (t + 1) * P]
            uu, inv = np.unique(lt, return_inverse=True)
            if len(uu) > U:
                return None
            uniq[: len(uu), t] = uu
            uniq[len(uu) :, t] = uu[0]
            mall[t * P : (t + 1) * P] = inv.astype(np.float32)
        in_maps.append(
            {
                "x": np.ascontiguousarray(x[order[sl]]),
                "chi": chi,
                "uniq": uniq,
                "mall": np.ascontiguousarray(
                    np.broadcast_to(mall[None, :], (U, B_SHARD))
                ),
                "iota": np.arange(U, dtype=np.float32).reshape(U, 1),
            }
        )
    return in_maps


def _prep_v2(x, labels_i, centers):
    cb = centers.astype(ml_dtypes.bfloat16)
    in_maps = []
    for c in range(N_CORES):
        sl = slice(c * B_SHARD, (c + 1) * B_SHARD)
        lab = np.ascontiguousarray(
            labels_i[sl].astype(np.int32).reshape(N_TILES, P).T
        )
        in_maps.append({"x": np.ascontiguousarray(x[sl]), "labels": lab, "cb": cb})
    return in_maps


_BUILDERS = {"v7": _build_v7, "v5": _build_v5, "v2": _build_v2}
_PREPPERS = {"v7": _prep_v7, "v5": _prep_v5, "v2": _prep_v2}


def _run(x, labels, centers, trace=False, force=None):
    x = np.ascontiguousarray(np.asarray(x), dtype=np.float32)
    labels_i = np.ascontiguousarray(np.asarray(labels)).astype(np.int64)
    centers = np.ascontiguousarray(np.asarray(centers), dtype=np.float32)
    assert x.shape == (BATCH, FEAT), x.shape
    assert labels_i.shape == (BATCH,), labels_i.shape
    assert centers.shape == (NUM_CLASSES, FEAT), centers.shape

    in_maps = None
    for variant in [force] if force else ["v7", "v5", "v2"]:
        in_maps = _PREPPERS[variant](x, labels_i, centers)
        if in_maps is not None:
            break
    assert in_maps is not None, "no kernel variant applicable"

    if variant not in _CACHE:
        _CACHE[variant] = _BUILDERS[variant]()
    nc = _CACHE[variant]

    res = run_bass_kernel_spmd(nc, in_maps, core_ids=list(range(N_CORES)), trace=trace)
    total = 0.0
    for c in range(N_CORES):
        total += float(res.results[c]["out"].astype(np.float64).sum())
    val = np.float32(total / 2.0 / BATCH)
    return val, res


def kernel(x, labels, centers):
    val, _ = _run(x, labels, centers)
    return val
